# revision 30
# baseline (speedup 1.0000x reference)
"""Trainium2 Bass kernel for the conditioned WaveNet denoiser.

Distribution strategy (8 NeuronCores):
  - Data-parallel over batch: core b owns sample b end-to-end (block loop +
    output head), with the small weights replicated.
  - The huge stacked conditioning Dense weights Dt/Ds are channel-sharded 8
    ways and shipped as fp8 (e3m4, scaled x32): core j computes the
    conditioning planes for ALL batches over its 16 channels, then a chunked
    AllToAll (fp8 payload, half the bytes of bf16) routes each batch's planes
    to its owner core, overlapped with the residual-block compute.
  - A tiny warmup AllToAll fires first thing to absorb the collectives
    bootstrap/barrier cost off the critical path.
  - The residual chain runs in f32 end-to-end: conv matmuls stream h/g as
    float32r (full PE rate at N=512), so no bf16 casts and better precision.
    The conditioning plane is added into PSUM with an extra (I/8) matmul so
    the tanh/sigmoid activations read PSUM directly.
  - AllToAll triggers are the only gpsimd-queue users in the hot window so
    the chain runs back-to-back; plane loads ride the vector queue.

kernel() accepts the FULL inputs and returns the FULL [8, 2048, 1] output.
"""

import os
import sys

import numpy as np

for _p in ("/opt/trn_rl_repo",):
    if _p not in sys.path and os.path.isdir(_p):
        sys.path.insert(0, _p)

import ml_dtypes  # noqa: E402

import concourse.bass as bass  # noqa: E402
import concourse.tile as tile  # noqa: E402
from concourse import bacc, bass_utils, mybir  # noqa: E402

# Problem constants (hardcoded per the spec; kernel.py must be self-contained).
L = 10
DILATIONS = [1, 2, 4, 8, 16, 32, 64, 128, 256, 512]
T = 2048
C = 128
COND = 16
B = 8
NCORES = 8
TS = 512          # time-tile (matmul moving N / one PSUM bank of f32)
PAD = 512         # zero padding around h: f32r matmuls need even/aligned APs
SPAD = 8          # zero padding around skip_sum for the W1 taps
NT = T // TS      # 4 time tiles
# AllToAll chunk sizes in lb units (lb = 2*l + branch); block l uses lbs
# (2l, 2l+1) so boundaries must be even.
CHUNKS = [4, 4, 4, 4, 4]
assert sum(CHUNKS) == 2 * L and all(c % 2 == 0 for c in CHUNKS)

F32 = mybir.dt.float32
U32 = mybir.dt.uint32
F32R = mybir.dt.float32r
BF16 = mybir.dt.bfloat16
FP8 = mybir.dt.float8e3
BF = ml_dtypes.bfloat16
F8 = ml_dtypes.float8_e3m4

STG_SCALE = 8.0      # stg = plane * 8 in e3m4 (fp8 only on the A2A payload)
IDENT_SCALE = 0.125  # consumer identity matmul un-scales the plane

AF = mybir.ActivationFunctionType
ALU = mybir.AluOpType


def _r(ap):
    """View an f32 AP as float32r for full-rate PE matmuls."""
    return ap.bitcast(F32R)


def _tap_range(t0, n, off):
    """Valid (out_lo, length) of an out tile [t0, t0+n) for input offset off."""
    lo = max(t0, -off)
    hi = min(t0 + n, T - off)
    return lo - t0, max(0, hi - lo)


def _chunk_of_lb(lb):
    acc = 0
    for ci, n in enumerate(CHUNKS):
        if lb < acc + n:
            return ci, lb - acc
        acc += n
    raise AssertionError


def _build_nc(has_p: bool, has_bres: bool, has_bskip: bool):
    nc = bacc.Bacc(
        "TRN2",
        target_bir_lowering=False,
        debug=False,
        num_devices=NCORES,
    )

    # ---- I/O declarations (per-core values supplied via in_maps) ----
    xw = nc.dram_tensor("xw", [1, T], F32R, kind="ExternalInput")
    wcT = nc.dram_tensor("wcT", [1, C], F32R, kind="ExternalInput")
    bcp = nc.dram_tensor("bcp", [C, 1], F32, kind="ExternalInput")
    cstat = nc.dram_tensor("cstat", [C, 64], BF16, kind="ExternalInput")
    ident8 = nc.dram_tensor("ident8", [C, C], FP8, kind="ExternalInput")
    # [pair, p, j, hh, t]: two lbs per slab so one producer DMA moves 2MB
    dtp = nc.dram_tensor("dtp", [L, C, 2, 2, T], BF16, kind="ExternalInput")
    wtp = nc.dram_tensor("wtp", [C, 6 * L, C], F32R, kind="ExternalInput")
    wsr = nc.dram_tensor("wsr", [C, 2 * L, C], F32R, kind="ExternalInput")
    w1p = nc.dram_tensor("w1p", [C, 3, 2048], F32R, kind="ExternalInput")
    b1p = nc.dram_tensor("b1p", [C, 16], F32, kind="ExternalInput")
    w2p = nc.dram_tensor("w2p", [C, 96, C], BF16, kind="ExternalInput")
    b2p = nc.dram_tensor("b2p", [C, 2], F32, kind="ExternalInput")
    w3p = nc.dram_tensor("w3p", [C, 2], BF16, kind="ExternalInput")
    b3p = nc.dram_tensor("b3p", [1, 1], F32, kind="ExternalInput")
    if has_p:
        ptp = nc.dram_tensor("ptp", [2 * L, 16, T], BF16, kind="ExternalInput")
        pstat = nc.dram_tensor("pstat", [8, 64], BF16, kind="ExternalInput")
    if has_bres:
        bresp = nc.dram_tensor("bresp", [C, L], F32, kind="ExternalInput")
    if has_bskip:
        bskips = nc.dram_tensor("bskips", [C, 1], F32, kind="ExternalInput")
    out = nc.dram_tensor("out", [1, T], F32, kind="ExternalOutput")

    rg = [list(range(NCORES))]

    with tile.TileContext(nc) as tc:
        with (
            tc.tile_pool(name="consts", bufs=1) as consts,
            tc.tile_pool(name="skipbuf", bufs=1) as skipbuf,
            tc.tile_pool(name="headw", bufs=1) as headw,
            tc.tile_pool(name="dram", bufs=1, space="DRAM") as dram,
        ):
            # ---- small constants (hot path first, sync queue) ----
            x_sb = consts.tile([1, T], F32R)
            nc.sync.dma_start(x_sb[:], xw[:, :])
            wc_sb = consts.tile([1, C], F32R)
            nc.sync.dma_start(wc_sb[:], wcT[:, :])
            bc_sb = consts.tile([C, 1], F32)
            nc.sync.dma_start(bc_sb[:], bcp[:, :])
            cs_sb = consts.tile([C, 64], BF16)
            nc.sync.dma_start(cs_sb[:], cstat[:, :])
            id8_sb = consts.tile([C, C], FP8)
            nc.sync.dma_start(id8_sb[:], ident8[:, :])
            if has_p:
                ps_sb = consts.tile([8, 64], BF16)
                nc.sync.dma_start(ps_sb[:], pstat[:, :])
            if has_bres:
                bres_sb = consts.tile([C, L], F32)
                nc.sync.dma_start(bres_sb[:], bresp[:, :])
            if has_bskip:
                bsk_sb = consts.tile([C, 1], F32)
                nc.sync.dma_start(bsk_sb[:], bskips[:, :])

            # Head weights load early (sync queue, behind dtp tiles); they
            # persist in their own pool until the head runs.
            w1_sb = headw.tile([C, 3, 2048], F32R)
            b1_sb = headw.tile([C, 16], F32)
            w2_sb = headw.tile([C, 96, C], BF16)
            b2_sb = headw.tile([C, 2], F32)
            w3_sb = headw.tile([C, 2], BF16)
            b3_sb = headw.tile([1, 1], F32)

            # AllToAll bounce buffers, one pair per chunk.
            a2a_in = []
            a2a_out = []
            for ci, nlb in enumerate(CHUNKS):
                ain = dram.tile([B, nlb, 16, T], FP8, name=f"a2a_in{ci}")
                aout = dram.tile([B, nlb, 16, T], FP8, name=f"a2a_out{ci}")
                a2a_in.append(ain)
                a2a_out.append(aout)

            with (
                tc.tile_pool(name="wbuf", bufs=1) as wbuf,
                tc.tile_pool(name="hbuf", bufs=2) as hbuf,
                tc.tile_pool(name="gbuf", bufs=3) as gbuf,
                tc.tile_pool(name="dtbuf", bufs=2) as dtbuf,
                tc.tile_pool(name="ptbuf", bufs=2) as ptbuf,
                tc.tile_pool(name="stgbuf", bufs=2) as stgbuf,
                tc.tile_pool(name="ttbuf", bufs=3) as ttbuf,
                tc.tile_pool(name="gtmp", bufs=3) as gtmp,
                tc.tile_pool(name="psum_prod", bufs=2, space="PSUM") as psum_prod,
                tc.tile_pool(name="psum_z", bufs=3, space="PSUM") as psum_z,
                tc.tile_pool(name="psum_sr", bufs=3, space="PSUM") as psum_sr,
            ):
                wt_sb = wbuf.tile([C, 6 * L, C], F32R)
                nc.sync.dma_start(wt_sb[:], wtp[:, :, :])
                wsr_sb = wbuf.tile([C, 2 * L, C], F32R)
                nc.sync.dma_start(wsr_sb[:], wsr[:, :, :])

                # ---- h = x * Wc + bc  (K=1 f32r matmul + biased copy) ----
                h = hbuf.tile([C, T + 2 * PAD], F32R, name="h0")
                nc.vector.memset(h[:, 0:PAD].bitcast(U32), 0)
                nc.vector.memset(h[:, PAD + T :].bitcast(U32), 0)
                for it in range(NT):
                    ph = psum_z.tile([C, TS], F32, name="pz", tag="z")
                    nc.tensor.matmul(
                        ph[:],
                        wc_sb[:, :],
                        x_sb[:, bass.ts(it, TS)],
                        start=True,
                        stop=True,
                    )
                    nc.scalar.activation(
                        h[:, PAD + it * TS : PAD + (it + 1) * TS],
                        ph[:],
                        AF.Identity,
                        bias=bc_sb[:, 0:1],
                    )

                # ---- producer: conditioning planes + chunked AllToAll ----
                lb = 0
                last_prod_mm = None
                for cki, nlb in enumerate(CHUNKS):
                    for lbc in range(nlb):
                        # one 2MB DMA per lb-pair amortizes the HWDGE fixed
                        # cost; the pair tile then feeds 16 matmuls
                        if lb % 2 == 0:
                            dpair = dtbuf.tile([C, 2, 2, T], BF16, name="dpair")
                            nc.sync.dma_start(dpair[:], dtp[lb // 2])
                        j = lb % 2
                        if has_p:
                            pt = ptbuf.tile([16, T], BF16, name="pt")
                            nc.sync.dma_start(pt[:], ptp[lb])
                        stg = stgbuf.tile([C, T], FP8, name="stg")
                        for it in range(NT):
                            ppr = psum_prod.tile([C, TS], F32, name="ppr")
                            tsl = bass.ts(it, TS)
                            for hh in range(2):
                                rows = slice(64 * hh, 64 * hh + 64)
                                last_prod_mm = nc.tensor.matmul(
                                    ppr[rows, :],
                                    cs_sb[:, :],
                                    dpair[:, j, hh, tsl],
                                    start=True,
                                    stop=not has_p,
                                )
                                if has_p:
                                    nc.tensor.matmul(
                                        ppr[rows, :],
                                        ps_sb[:, :],
                                        pt[8 * hh : 8 * hh + 8, tsl],
                                        start=False,
                                        stop=True,
                                    )
                            nc.vector.tensor_scalar_mul(
                                stg[:, tsl], ppr[:], STG_SCALE
                            )
                        # stg stores ride the scalar queue: their semaphore
                        # waits (on the producer matmuls) must not block the
                        # dt2 loads behind them in the sync HWDGE FIFO.
                        for hh in range(2):
                            nc.scalar.dma_start(
                                a2a_in[cki][:, lbc, 8 * hh : 8 * hh + 8, :],
                                stg[64 * hh : 64 * hh + 64, :],
                            )
                        lb += 1
                    last_cc = nc.gpsimd.collective_compute(
                        "AllToAll",
                        ALU.bypass,
                        replica_groups=rg,
                        ins=[a2a_in[cki][:, :, :, :].opt()],
                        outs=[a2a_out[cki][:, :, :, :].opt()],
                    )

                # Head weights ride the gpsimd queue pinned behind the last
                # collective trigger so the scheduler cannot hoist them ahead
                # of the chain; transfers land mid-kernel, well before the
                # head needs them.
                for _dst, _src in (
                    (w1_sb, w1p), (b1_sb, b1p), (w2_sb, w2p),
                    (b2_sb, b2p), (w3_sb, w3p), (b3_sb, b3p),
                ):
                    _dma = nc.gpsimd.dma_start(_dst[:], _src[:])
                    tile.add_dep_helper(
                        _dma.ins, last_cc.ins, reason="head weights after chain"
                    )

                # ---- residual block loop (consumer) ----
                # Keep the in-order PE stream strictly producer-first: a
                # consumer matmul scheduled between producer matmuls would
                # head-of-line-block them while waiting for its AllToAll.
                prod_fence = last_prod_mm
                skip_sb = skipbuf.tile([C, SPAD + T + SPAD], F32R, name="skip")
                nc.vector.memset(skip_sb[:, 0:SPAD].bitcast(U32), 0)
                nc.vector.memset(skip_sb[:, SPAD + T :].bitcast(U32), 0)
                for l in range(L):
                    d = DILATIONS[l]
                    cki, lbc0 = _chunk_of_lb(2 * l)
                    planes = []
                    for br in range(2):
                        tb = ttbuf.tile([C, T], FP8, name="tb")
                        nc.scalar.dma_start(
                            tb[:], a2a_out[cki][:, lbc0 + br, :, :]
                        )
                        planes.append(tb)
                    g_tiles = {}
                    h_new = hbuf.tile([C, T + 2 * PAD], F32R, name="hn")
                    nc.vector.memset(h_new[:, 0:PAD].bitcast(U32), 0)
                    nc.vector.memset(h_new[:, PAD + T :].bitcast(U32), 0)

                    def branch_mms(it):
                        nonlocal prod_fence
                        t0 = it * TS
                        tsl = bass.ts(it, TS)
                        avs = []
                        for br, fn in ((0, AF.Tanh), (1, AF.Sigmoid)):
                            pz = psum_z.tile([C, TS], F32, name="pz", tag="z")
                            # plane/8 added on the PE via (I/8) matmul so the
                            # activation reads PSUM directly (the DVE is the
                            # block phase's scarce engine, not the PE)
                            mm = nc.tensor.matmul(
                                pz[:],
                                id8_sb[:, :],
                                planes[br][:, tsl],
                                start=True,
                                stop=False,
                            )
                            if prod_fence is not None:
                                tile.add_dep_helper(
                                    mm.ins,
                                    prod_fence.ins,
                                    reason="consumer after producer",
                                )
                                prod_fence = None
                            for idx, (tap, off) in enumerate(
                                ((1, 0), (0, -d), (2, d))
                            ):
                                w_ap = wt_sb[:, (l * 2 + br) * 3 + tap, :]
                                nc.tensor.matmul(
                                    pz[:],
                                    w_ap,
                                    h[:, PAD + t0 + off : PAD + t0 + off + TS],
                                    start=False,
                                    stop=idx == 2,
                                )
                            av = gtmp.tile([C, TS], F32, name="av", tag="av")
                            nc.scalar.activation(av[:], pz[:], fn)
                            avs.append(av)
                        gt = gbuf.tile([C, TS], F32R, name="g")
                        g_tiles[it] = gt
                        nc.vector.tensor_mul(gt[:], avs[0][:], avs[1][:])

                    def skip_res_mms(it):
                        gt = g_tiles[it]
                        psk = psum_sr.tile([C, TS], F32, name="psk", tag="sr")
                        nc.tensor.matmul(
                            psk[:],
                            wsr_sb[:, 2 * l, :],
                            gt[:],
                            start=True,
                            stop=True,
                        )
                        ssl = slice(SPAD + it * TS, SPAD + (it + 1) * TS)
                        if l == 0:
                            nc.vector.tensor_copy(skip_sb[:, ssl], psk[:])
                        else:
                            nc.vector.tensor_add(
                                skip_sb[:, ssl], skip_sb[:, ssl], psk[:]
                            )
                        prs = psum_sr.tile([C, TS], F32, name="prs", tag="sr")
                        nc.tensor.matmul(
                            prs[:],
                            wsr_sb[:, 2 * l + 1, :],
                            gt[:],
                            start=True,
                            stop=True,
                        )
                        hsl = slice(PAD + it * TS, PAD + (it + 1) * TS)
                        if has_bres:
                            nc.vector.scalar_tensor_tensor(
                                h_new[:, hsl],
                                prs[:],
                                bres_sb[:, l : l + 1],
                                h[:, hsl],
                                ALU.add,
                                ALU.add,
                            )
                        else:
                            nc.vector.tensor_add(h_new[:, hsl], prs[:], h[:, hsl])

                    # one-tile software pipeline: branch mms run one tile
                    # ahead of skip/res so the PE never waits on the g chain
                    branch_mms(0)
                    for it in range(1, NT):
                        branch_mms(it)
                        skip_res_mms(it - 1)
                    skip_res_mms(NT - 1)
                    h = h_new

                if has_bskip:
                    nc.scalar.activation(
                        skip_sb[:, SPAD : SPAD + T],
                        skip_sb[:, SPAD : SPAD + T],
                        AF.Identity,
                        bias=bsk_sb[:, 0:1],
                    )

            # ---- output head ----
            with (
                tc.tile_pool(name="o1buf", bufs=1) as o1buf,
                tc.tile_pool(name="o2buf", bufs=1) as o2buf,
                tc.tile_pool(name="obuf", bufs=1) as obuf,
                tc.tile_pool(name="psum_h1", bufs=4, space="PSUM") as psum_h1,
                tc.tile_pool(name="psum_h2", bufs=2, space="PSUM") as psum_h2,
                tc.tile_pool(name="psum_h3", bufs=2, space="PSUM") as psum_h3,
            ):
                out1 = o1buf.tile([C, 16, T], BF16, name="out1")
                out2 = o2buf.tile([C, 2, T], BF16, name="out2")
                o_sb = obuf.tile([1, T], F32, name="o_sb")
                for oc in range(16):
                    for it in range(NT):
                        t0 = it * TS
                        p1 = psum_h1.tile([C, TS], F32, name="p1")
                        for idx, (tap, off) in enumerate(
                            ((1, 0), (0, -1), (2, 1))
                        ):
                            w_ap = w1_sb[:, tap, oc * C : (oc + 1) * C]
                            nc.tensor.matmul(
                                p1[:],
                                w_ap,
                                skip_sb[:, SPAD + t0 + off : SPAD + t0 + off + TS],
                                start=idx == 0,
                                stop=idx == 2,
                            )
                        # bias + relu fused on DVE (ACT would be the head's
                        # bottleneck at ~0.7us/op)
                        nc.vector.tensor_scalar(
                            out1[:, oc, bass.ts(it, TS)],
                            p1[:],
                            b1_sb[:, oc : oc + 1],
                            0.0,
                            ALU.add,
                            ALU.max,
                        )
                for oc2 in range(2):
                    for it in range(NT):
                        t0 = it * TS
                        p2 = psum_h2.tile([C, TS], F32, name="p2")
                        taps = []
                        for tap, off in ((1, 0), (0, -1), (2, 1)):
                            lo, n = _tap_range(t0, TS, off)
                            if n > 0:
                                taps.append((tap, off, lo, n))
                        nmm = len(taps) * 16
                        k = 0
                        for tap, off, lo, n in taps:
                            for ic in range(16):
                                w_ap = w2_sb[:, (tap * 16 + ic) * 2 + oc2, :]
                                nc.tensor.matmul(
                                    p2[:, lo : lo + n],
                                    w_ap,
                                    out1[:, ic, t0 + lo + off : t0 + lo + off + n],
                                    start=k == 0,
                                    stop=k == nmm - 1,
                                )
                                k += 1
                        nc.vector.tensor_scalar(
                            out2[:, oc2, bass.ts(it, TS)],
                            p2[:],
                            b2_sb[:, oc2 : oc2 + 1],
                            0.0,
                            ALU.add,
                            ALU.max,
                        )
                for it in range(NT):
                    tsl = bass.ts(it, TS)
                    p3 = psum_h3.tile([1, TS], F32, name="p3")
                    for ic in range(2):
                        nc.tensor.matmul(
                            p3[:],
                            w3_sb[:, ic : ic + 1],
                            out2[:, ic, tsl],
                            start=ic == 0,
                            stop=ic == 1,
                        )
                    nc.scalar.activation(
                        o_sb[:, tsl], p3[:], AF.Tanh, bias=b3_sb[:, 0:1]
                    )
                nc.sync.dma_start(out[:, :], o_sb[:])

    nc.compile()
    return nc


_NC_CACHE = {}


def _get_nc(has_p, has_bres, has_bskip):
    key = (has_p, has_bres, has_bskip)
    if key not in _NC_CACHE:
        _NC_CACHE[key] = _build_nc(*key)
    return _NC_CACHE[key]


def _to_f8(a):
    return np.clip(a, -15.5, 15.5).astype(F8)


def _pack_inputs(
    x, condition, Wc, bc, Wt, bt, Ws, bs, Dt, Bt, Ds, Bs,
    Wskip, bskip, Wres, bres, W1, b1, W2, b2, W3, b3,
):
    """Host-side sharding + layout packs. Returns (in_maps, flags)."""
    f32 = np.float32
    x = np.asarray(x, f32)
    condition = np.asarray(condition, f32)
    has_p = bool(
        np.any(np.asarray(Bt)) or np.any(np.asarray(Bs))
        or np.any(np.asarray(bt)) or np.any(np.asarray(bs))
    )
    has_bres = bool(np.any(np.asarray(bres)))
    has_bskip = bool(np.any(np.asarray(bskip)))

    # dtp: [core, lb=2l+br, p=16g+c, hh, t] = D_br[l, c, t, 16j+8hh+g]
    D = np.stack([np.asarray(Dt, f32), np.asarray(Ds, f32)], axis=1)
    D = D.reshape(L, 2, COND, T, 8, 2, 8)
    dtp_all = np.ascontiguousarray(
        D.transpose(4, 0, 1, 6, 2, 5, 3).reshape(NCORES, L, 2, C, 2, T)
        .transpose(0, 1, 3, 2, 4, 5)
    ).astype(BF)
    del D

    # cstat: [16g+c, 8b+g] = condition[b, c]
    cstat = np.zeros((C, 64), f32)
    for g in range(8):
        cstat[16 * g : 16 * g + 16, g::8] = condition.T
    cstat = cstat.astype(BF)

    # wtp: [cin, (l,br,tap), cout] f32
    Wg = np.stack([np.asarray(Wt, f32), np.asarray(Ws, f32)], axis=1)
    wtp = np.ascontiguousarray(Wg.transpose(3, 0, 1, 2, 4).reshape(C, 6 * L, C))
    # wsr: [cin, (l, skip/res), cout] f32
    Ssr = np.stack([np.asarray(Wskip, f32)[:, 0], np.asarray(Wres, f32)[:, 0]], axis=1)
    wsr = np.ascontiguousarray(Ssr.transpose(2, 0, 1, 3).reshape(C, 2 * L, C))

    w1p = np.ascontiguousarray(np.asarray(W1, f32).transpose(1, 0, 2))
    b1p = np.ascontiguousarray(np.asarray(b1, f32).reshape(16, C).T)
    w2p = np.ascontiguousarray(
        np.asarray(W2, f32).reshape(3, 16, C, 2, C).transpose(2, 0, 1, 3, 4)
        .reshape(C, 96, C)
    ).astype(BF)
    b2p = np.ascontiguousarray(np.asarray(b2, f32).reshape(2, C).T)
    w3p = np.ascontiguousarray(np.asarray(W3, f32)[0, :, 0].reshape(2, C).T).astype(BF)
    b3p = np.asarray(b3, f32).reshape(1, 1)
    wcT = np.ascontiguousarray(np.asarray(Wc, f32).reshape(1, C))
    ident8 = (np.eye(C, dtype=f32) * IDENT_SCALE).astype(F8)
    bcp = np.asarray(bc, f32).reshape(C, 1)

    base = {
        "wcT": wcT, "bcp": bcp, "cstat": cstat, "wtp": wtp, "wsr": wsr,
        "w1p": w1p, "b1p": b1p, "w2p": w2p, "b2p": b2p, "w3p": w3p,
        "b3p": b3p, "ident8": ident8,
    }
    if has_p:
        P = np.stack(
            [
                np.asarray(Bt, f32) + np.asarray(bt, f32)[:, None, :],
                np.asarray(Bs, f32) + np.asarray(bs, f32)[:, None, :],
            ],
            axis=1,
        )  # [L, 2, T, C]
        P = P.reshape(L, 2, T, 8, 2, 8)
        ptp_all = np.ascontiguousarray(
            P.transpose(3, 0, 1, 4, 5, 2).reshape(NCORES, 2 * L, 16, T)
        ).astype(BF)
        del P
        pstat = np.zeros((8, 64), f32)
        for g in range(8):
            pstat[g, g::8] = 1.0
        base["pstat"] = pstat.astype(BF)
    if has_bres:
        base["bresp"] = np.ascontiguousarray(np.asarray(bres, f32).T)
    if has_bskip:
        base["bskips"] = np.asarray(bskip, f32).sum(axis=0).reshape(C, 1)

    in_maps = []
    for j in range(NCORES):
        m = dict(base)
        m["xw"] = np.ascontiguousarray(x[j, :, 0].reshape(1, T))
        m["dtp"] = dtp_all[j]
        if has_p:
            m["ptp"] = ptp_all[j]
        in_maps.append(m)
    return in_maps, (has_p, has_bres, has_bskip)


def kernel(**inputs) -> np.ndarray:
    in_maps, flags = _pack_inputs(**inputs)
    nc = _get_nc(*flags)
    res = bass_utils.run_bass_kernel_spmd(
        nc, in_maps, core_ids=list(range(NCORES))
    )
    outs = [res.results[j]["out"].reshape(T, 1) for j in range(NCORES)]
    return np.stack(outs, axis=0).astype(np.float32)


# revision 31
# speedup vs baseline: 1.1431x; 1.1431x over previous
"""Trainium2 Bass kernel for the conditioned WaveNet denoiser.

Distribution strategy (8 NeuronCores):
  - Data-parallel over batch: core b owns sample b end-to-end (block loop +
    output head), with the small weights replicated.
  - The huge stacked conditioning Dense weights Dt/Ds are channel-sharded 8
    ways and shipped as fp8 (e3m4, scaled x32): core j computes the
    conditioning planes for ALL batches over its 16 channels, then a chunked
    AllToAll (fp8 payload, half the bytes of bf16) routes each batch's planes
    to its owner core, overlapped with the residual-block compute.
  - A tiny warmup AllToAll fires first thing to absorb the collectives
    bootstrap/barrier cost off the critical path.
  - The residual chain runs in f32 end-to-end: conv matmuls stream h/g as
    float32r (full PE rate at N=512), so no bf16 casts and better precision.
    The conditioning plane is added into PSUM with an extra (I/8) matmul so
    the tanh/sigmoid activations read PSUM directly.
  - AllToAll triggers are the only gpsimd-queue users in the hot window so
    the chain runs back-to-back; plane loads ride the vector queue.

kernel() accepts the FULL inputs and returns the FULL [8, 2048, 1] output.
"""

import os
import sys

import numpy as np

for _p in ("/opt/trn_rl_repo",):
    if _p not in sys.path and os.path.isdir(_p):
        sys.path.insert(0, _p)

import ml_dtypes  # noqa: E402

import concourse.bass as bass  # noqa: E402
import concourse.tile as tile  # noqa: E402
from concourse import bacc, bass_utils, mybir  # noqa: E402

# Problem constants (hardcoded per the spec; kernel.py must be self-contained).
L = 10
DILATIONS = [1, 2, 4, 8, 16, 32, 64, 128, 256, 512]
T = 2048
C = 128
COND = 16
B = 8
NCORES = 8
TS = 512          # time-tile (matmul moving N / one PSUM bank of f32)
PAD = 512         # zero padding around h: f32r matmuls need even/aligned APs
SPAD = 8          # zero padding around skip_sum for the W1 taps
NT = T // TS      # 4 time tiles
# AllToAll chunk sizes in lb units (lb = 2*l + branch); block l uses lbs
# (2l, 2l+1) so boundaries must be even.
CHUNKS = [4, 4, 4, 4, 4]
assert sum(CHUNKS) == 2 * L and all(c % 2 == 0 for c in CHUNKS)

F32 = mybir.dt.float32
U32 = mybir.dt.uint32
F32R = mybir.dt.float32r
BF16 = mybir.dt.bfloat16
FP8 = mybir.dt.float8e3
BF = ml_dtypes.bfloat16
F8 = ml_dtypes.float8_e3m4

STG_SCALE = 8.0      # stg = plane * 8 in e3m4 (fp8 only on the A2A payload)
IDENT_SCALE = 0.125  # consumer identity matmul un-scales the plane

AF = mybir.ActivationFunctionType
ALU = mybir.AluOpType


def _r(ap):
    """View an f32 AP as float32r for full-rate PE matmuls."""
    return ap.bitcast(F32R)


def _tap_range(t0, n, off):
    """Valid (out_lo, length) of an out tile [t0, t0+n) for input offset off."""
    lo = max(t0, -off)
    hi = min(t0 + n, T - off)
    return lo - t0, max(0, hi - lo)


def _chunk_of_lb(lb):
    acc = 0
    for ci, n in enumerate(CHUNKS):
        if lb < acc + n:
            return ci, lb - acc
        acc += n
    raise AssertionError


def _build_nc(has_p: bool, has_bres: bool, has_bskip: bool):
    nc = bacc.Bacc(
        "TRN2",
        target_bir_lowering=False,
        debug=False,
        num_devices=NCORES,
    )

    # ---- I/O declarations (per-core values supplied via in_maps) ----
    xw = nc.dram_tensor("xw", [1, T], F32R, kind="ExternalInput")
    wcT = nc.dram_tensor("wcT", [1, C], F32R, kind="ExternalInput")
    bcp = nc.dram_tensor("bcp", [C, 1], F32, kind="ExternalInput")
    cstat = nc.dram_tensor("cstat", [C, 64], BF16, kind="ExternalInput")
    ident8 = nc.dram_tensor("ident8", [C, C], FP8, kind="ExternalInput")
    # [lb, p, hh, t] so one plane-pair is a single contiguous-line DMA
    dtp = nc.dram_tensor("dtp", [2 * L, C, 2, T], BF16, kind="ExternalInput")
    wtp = nc.dram_tensor("wtp", [C, 6 * L, C], F32R, kind="ExternalInput")
    wsr = nc.dram_tensor("wsr", [C, 2 * L, C], F32R, kind="ExternalInput")
    w1p = nc.dram_tensor("w1p", [C, 3, 2048], F32R, kind="ExternalInput")
    b1p = nc.dram_tensor("b1p", [C, 16], F32, kind="ExternalInput")
    w2p = nc.dram_tensor("w2p", [C, 96, C], BF16, kind="ExternalInput")
    b2p = nc.dram_tensor("b2p", [C, 2], F32, kind="ExternalInput")
    w3p = nc.dram_tensor("w3p", [C, 2], BF16, kind="ExternalInput")
    b3p = nc.dram_tensor("b3p", [1, 1], F32, kind="ExternalInput")
    if has_p:
        ptp = nc.dram_tensor("ptp", [2 * L, 16, T], BF16, kind="ExternalInput")
        pstat = nc.dram_tensor("pstat", [8, 64], BF16, kind="ExternalInput")
    if has_bres:
        bresp = nc.dram_tensor("bresp", [C, L], F32, kind="ExternalInput")
    if has_bskip:
        bskips = nc.dram_tensor("bskips", [C, 1], F32, kind="ExternalInput")
    out = nc.dram_tensor("out", [1, T], F32, kind="ExternalOutput")

    rg = [list(range(NCORES))]

    with tile.TileContext(nc) as tc:
        with (
            tc.tile_pool(name="consts", bufs=1) as consts,
            tc.tile_pool(name="skipbuf", bufs=1) as skipbuf,
            tc.tile_pool(name="headw", bufs=1) as headw,
            tc.tile_pool(name="dram", bufs=1, space="DRAM") as dram,
        ):
            # ---- small constants (hot path first, sync queue) ----
            x_sb = consts.tile([1, T], F32R)
            nc.sync.dma_start(x_sb[:], xw[:, :])
            wc_sb = consts.tile([1, C], F32R)
            nc.sync.dma_start(wc_sb[:], wcT[:, :])
            bc_sb = consts.tile([C, 1], F32)
            nc.sync.dma_start(bc_sb[:], bcp[:, :])
            cs_sb = consts.tile([C, 64], BF16)
            nc.sync.dma_start(cs_sb[:], cstat[:, :])
            id8_sb = consts.tile([C, C], FP8)
            nc.sync.dma_start(id8_sb[:], ident8[:, :])
            if has_p:
                ps_sb = consts.tile([8, 64], BF16)
                nc.sync.dma_start(ps_sb[:], pstat[:, :])
            if has_bres:
                bres_sb = consts.tile([C, L], F32)
                nc.sync.dma_start(bres_sb[:], bresp[:, :])
            if has_bskip:
                bsk_sb = consts.tile([C, 1], F32)
                nc.sync.dma_start(bsk_sb[:], bskips[:, :])

            # Head weights load early (sync queue, behind dtp tiles); they
            # persist in their own pool until the head runs.
            w1_sb = headw.tile([C, 3, 2048], F32R)
            b1_sb = headw.tile([C, 16], F32)
            w2_sb = headw.tile([C, 96, C], BF16)
            b2_sb = headw.tile([C, 2], F32)
            w3_sb = headw.tile([C, 2], BF16)
            b3_sb = headw.tile([1, 1], F32)

            # AllToAll bounce buffers, one pair per chunk.
            a2a_in = []
            a2a_out = []
            for ci, nlb in enumerate(CHUNKS):
                ain = dram.tile([B, nlb, 16, T], FP8, name=f"a2a_in{ci}")
                aout = dram.tile([B, nlb, 16, T], FP8, name=f"a2a_out{ci}")
                a2a_in.append(ain)
                a2a_out.append(aout)

            with (
                tc.tile_pool(name="wbuf", bufs=1) as wbuf,
                tc.tile_pool(name="hbuf", bufs=2) as hbuf,
                tc.tile_pool(name="gbuf", bufs=4) as gbuf,
                tc.tile_pool(name="dtbuf", bufs=3) as dtbuf,
                tc.tile_pool(name="ptbuf", bufs=2) as ptbuf,
                tc.tile_pool(name="stgbuf", bufs=3) as stgbuf,
                tc.tile_pool(name="ttbuf", bufs=4) as ttbuf,
                tc.tile_pool(name="gtmp", bufs=4) as gtmp,
                tc.tile_pool(name="psum_prod", bufs=2, space="PSUM") as psum_prod,
                tc.tile_pool(name="psum_z", bufs=3, space="PSUM") as psum_z,
                tc.tile_pool(name="psum_sr", bufs=3, space="PSUM") as psum_sr,
            ):
                wt_sb = wbuf.tile([C, 6 * L, C], F32R)
                nc.sync.dma_start(wt_sb[:], wtp[:, :, :])
                wsr_sb = wbuf.tile([C, 2 * L, C], F32R)
                nc.sync.dma_start(wsr_sb[:], wsr[:, :, :])

                # ---- h = x * Wc + bc  (K=1 f32r matmul + biased copy) ----
                h = hbuf.tile([C, T + 2 * PAD], F32R, name="h0")
                nc.vector.memset(h[:, 0:PAD].bitcast(U32), 0)
                nc.vector.memset(h[:, PAD + T :].bitcast(U32), 0)
                for it in range(NT):
                    ph = psum_z.tile([C, TS], F32, name="pz", tag="z")
                    nc.tensor.matmul(
                        ph[:],
                        wc_sb[:, :],
                        x_sb[:, bass.ts(it, TS)],
                        start=True,
                        stop=True,
                    )
                    nc.scalar.activation(
                        h[:, PAD + it * TS : PAD + (it + 1) * TS],
                        ph[:],
                        AF.Identity,
                        bias=bc_sb[:, 0:1],
                    )

                # ---- producer: conditioning planes + chunked AllToAll ----
                lb = 0
                last_prod_mm = None
                for cki, nlb in enumerate(CHUNKS):
                    for lbc in range(nlb):
                        dt2 = dtbuf.tile([C, 2, T], BF16, name="dt2")
                        nc.sync.dma_start(dt2[:], dtp[lb])
                        if has_p:
                            pt = ptbuf.tile([16, T], BF16, name="pt")
                            nc.sync.dma_start(pt[:], ptp[lb])
                        stg = stgbuf.tile([C, T], FP8, name="stg")
                        for it in range(NT):
                            ppr = psum_prod.tile([C, TS], F32, name="ppr")
                            tsl = bass.ts(it, TS)
                            for hh in range(2):
                                rows = slice(64 * hh, 64 * hh + 64)
                                last_prod_mm = nc.tensor.matmul(
                                    ppr[rows, :],
                                    cs_sb[:, :],
                                    dt2[:, hh, tsl],
                                    start=True,
                                    stop=not has_p,
                                )
                                if has_p:
                                    nc.tensor.matmul(
                                        ppr[rows, :],
                                        ps_sb[:, :],
                                        pt[8 * hh : 8 * hh + 8, tsl],
                                        start=False,
                                        stop=True,
                                    )
                            nc.vector.tensor_scalar_mul(
                                stg[:, tsl], ppr[:], STG_SCALE
                            )
                        # stg stores ride the scalar queue: their semaphore
                        # waits (on the producer matmuls) must not block the
                        # dt2 loads behind them in the sync HWDGE FIFO.
                        for hh in range(2):
                            nc.scalar.dma_start(
                                a2a_in[cki][:, lbc, 8 * hh : 8 * hh + 8, :],
                                stg[64 * hh : 64 * hh + 64, :],
                            )
                        lb += 1
                    last_cc = nc.gpsimd.collective_compute(
                        "AllToAll",
                        ALU.bypass,
                        replica_groups=rg,
                        ins=[a2a_in[cki][:, :, :, :].opt()],
                        outs=[a2a_out[cki][:, :, :, :].opt()],
                    )

                # Head weights ride the gpsimd queue pinned behind the last
                # collective trigger so the scheduler cannot hoist them ahead
                # of the chain; transfers land mid-kernel, well before the
                # head needs them.
                for _dst, _src in (
                    (w1_sb, w1p), (b1_sb, b1p), (w2_sb, w2p),
                    (b2_sb, b2p), (w3_sb, w3p), (b3_sb, b3p),
                ):
                    _dma = nc.gpsimd.dma_start(_dst[:], _src[:])
                    tile.add_dep_helper(
                        _dma.ins, last_cc.ins, reason="head weights after chain"
                    )

                # ---- residual block loop (consumer) ----
                # Keep the in-order PE stream strictly producer-first: a
                # consumer matmul scheduled between producer matmuls would
                # head-of-line-block them while waiting for its AllToAll.
                prod_fence = last_prod_mm
                skip_sb = skipbuf.tile([C, SPAD + T + SPAD], F32R, name="skip")
                nc.vector.memset(skip_sb[:, 0:SPAD].bitcast(U32), 0)
                nc.vector.memset(skip_sb[:, SPAD + T :].bitcast(U32), 0)
                for l in range(L):
                    d = DILATIONS[l]
                    cki, lbc0 = _chunk_of_lb(2 * l)
                    planes = []
                    for br in range(2):
                        tb = ttbuf.tile([C, T], FP8, name="tb")
                        nc.scalar.dma_start(
                            tb[:], a2a_out[cki][:, lbc0 + br, :, :]
                        )
                        planes.append(tb)
                    g_tiles = {}
                    h_new = hbuf.tile([C, T + 2 * PAD], F32R, name="hn")
                    nc.vector.memset(h_new[:, 0:PAD].bitcast(U32), 0)
                    nc.vector.memset(h_new[:, PAD + T :].bitcast(U32), 0)

                    def branch_mms(it):
                        nonlocal prod_fence
                        t0 = it * TS
                        tsl = bass.ts(it, TS)
                        avs = []
                        for br, fn in ((0, AF.Tanh), (1, AF.Sigmoid)):
                            pz = psum_z.tile([C, TS], F32, name="pz", tag="z")
                            # plane/8 added on the PE via (I/8) matmul so the
                            # activation reads PSUM directly (the DVE is the
                            # block phase's scarce engine, not the PE)
                            mm = nc.tensor.matmul(
                                pz[:],
                                id8_sb[:, :],
                                planes[br][:, tsl],
                                start=True,
                                stop=False,
                            )
                            if prod_fence is not None:
                                tile.add_dep_helper(
                                    mm.ins,
                                    prod_fence.ins,
                                    reason="consumer after producer",
                                )
                                prod_fence = None
                            for idx, (tap, off) in enumerate(
                                ((1, 0), (0, -d), (2, d))
                            ):
                                w_ap = wt_sb[:, (l * 2 + br) * 3 + tap, :]
                                nc.tensor.matmul(
                                    pz[:],
                                    w_ap,
                                    h[:, PAD + t0 + off : PAD + t0 + off + TS],
                                    start=False,
                                    stop=idx == 2,
                                )
                            av = gtmp.tile([C, TS], F32, name="av", tag="av")
                            nc.scalar.activation(av[:], pz[:], fn)
                            avs.append(av)
                        gt = gbuf.tile([C, TS], F32R, name="g")
                        g_tiles[it] = gt
                        nc.vector.tensor_mul(gt[:], avs[0][:], avs[1][:])

                    def skip_res_mms(it):
                        gt = g_tiles[it]
                        psk = psum_sr.tile([C, TS], F32, name="psk", tag="sr")
                        nc.tensor.matmul(
                            psk[:],
                            wsr_sb[:, 2 * l, :],
                            gt[:],
                            start=True,
                            stop=True,
                        )
                        ssl = slice(SPAD + it * TS, SPAD + (it + 1) * TS)
                        if l == 0:
                            nc.vector.tensor_copy(skip_sb[:, ssl], psk[:])
                        else:
                            nc.vector.tensor_add(
                                skip_sb[:, ssl], skip_sb[:, ssl], psk[:]
                            )
                        prs = psum_sr.tile([C, TS], F32, name="prs", tag="sr")
                        nc.tensor.matmul(
                            prs[:],
                            wsr_sb[:, 2 * l + 1, :],
                            gt[:],
                            start=True,
                            stop=True,
                        )
                        hsl = slice(PAD + it * TS, PAD + (it + 1) * TS)
                        if has_bres:
                            nc.vector.scalar_tensor_tensor(
                                h_new[:, hsl],
                                prs[:],
                                bres_sb[:, l : l + 1],
                                h[:, hsl],
                                ALU.add,
                                ALU.add,
                            )
                        else:
                            nc.vector.tensor_add(h_new[:, hsl], prs[:], h[:, hsl])

                    # one-tile software pipeline: branch mms run one tile
                    # ahead of skip/res so the PE never waits on the g chain
                    branch_mms(0)
                    for it in range(1, NT):
                        branch_mms(it)
                        skip_res_mms(it - 1)
                    skip_res_mms(NT - 1)
                    h = h_new

                if has_bskip:
                    nc.scalar.activation(
                        skip_sb[:, SPAD : SPAD + T],
                        skip_sb[:, SPAD : SPAD + T],
                        AF.Identity,
                        bias=bsk_sb[:, 0:1],
                    )

            # ---- output head ----
            with (
                tc.tile_pool(name="o1buf", bufs=1) as o1buf,
                tc.tile_pool(name="o2buf", bufs=1) as o2buf,
                tc.tile_pool(name="obuf", bufs=1) as obuf,
                tc.tile_pool(name="psum_h1", bufs=4, space="PSUM") as psum_h1,
                tc.tile_pool(name="psum_h2", bufs=2, space="PSUM") as psum_h2,
                tc.tile_pool(name="psum_h3", bufs=2, space="PSUM") as psum_h3,
            ):
                out1 = o1buf.tile([C, 16, T], BF16, name="out1")
                out2 = o2buf.tile([C, 2, T], BF16, name="out2")
                o_sb = obuf.tile([1, T], F32, name="o_sb")
                for oc in range(16):
                    for it in range(NT):
                        t0 = it * TS
                        p1 = psum_h1.tile([C, TS], F32, name="p1")
                        for idx, (tap, off) in enumerate(
                            ((1, 0), (0, -1), (2, 1))
                        ):
                            w_ap = w1_sb[:, tap, oc * C : (oc + 1) * C]
                            nc.tensor.matmul(
                                p1[:],
                                w_ap,
                                skip_sb[:, SPAD + t0 + off : SPAD + t0 + off + TS],
                                start=idx == 0,
                                stop=idx == 2,
                            )
                        # bias + relu fused on DVE (ACT would be the head's
                        # bottleneck at ~0.7us/op)
                        nc.vector.tensor_scalar(
                            out1[:, oc, bass.ts(it, TS)],
                            p1[:],
                            b1_sb[:, oc : oc + 1],
                            0.0,
                            ALU.add,
                            ALU.max,
                        )
                for oc2 in range(2):
                    for it in range(NT):
                        t0 = it * TS
                        p2 = psum_h2.tile([C, TS], F32, name="p2")
                        taps = []
                        for tap, off in ((1, 0), (0, -1), (2, 1)):
                            lo, n = _tap_range(t0, TS, off)
                            if n > 0:
                                taps.append((tap, off, lo, n))
                        nmm = len(taps) * 16
                        k = 0
                        for tap, off, lo, n in taps:
                            for ic in range(16):
                                w_ap = w2_sb[:, (tap * 16 + ic) * 2 + oc2, :]
                                nc.tensor.matmul(
                                    p2[:, lo : lo + n],
                                    w_ap,
                                    out1[:, ic, t0 + lo + off : t0 + lo + off + n],
                                    start=k == 0,
                                    stop=k == nmm - 1,
                                )
                                k += 1
                        nc.vector.tensor_scalar(
                            out2[:, oc2, bass.ts(it, TS)],
                            p2[:],
                            b2_sb[:, oc2 : oc2 + 1],
                            0.0,
                            ALU.add,
                            ALU.max,
                        )
                for it in range(NT):
                    tsl = bass.ts(it, TS)
                    p3 = psum_h3.tile([1, TS], F32, name="p3")
                    for ic in range(2):
                        nc.tensor.matmul(
                            p3[:],
                            w3_sb[:, ic : ic + 1],
                            out2[:, ic, tsl],
                            start=ic == 0,
                            stop=ic == 1,
                        )
                    nc.scalar.activation(
                        o_sb[:, tsl], p3[:], AF.Tanh, bias=b3_sb[:, 0:1]
                    )
                nc.sync.dma_start(out[:, :], o_sb[:])

    nc.compile()
    return nc


_NC_CACHE = {}


def _get_nc(has_p, has_bres, has_bskip):
    key = (has_p, has_bres, has_bskip)
    if key not in _NC_CACHE:
        _NC_CACHE[key] = _build_nc(*key)
    return _NC_CACHE[key]


def _to_f8(a):
    return np.clip(a, -15.5, 15.5).astype(F8)


def _pack_inputs(
    x, condition, Wc, bc, Wt, bt, Ws, bs, Dt, Bt, Ds, Bs,
    Wskip, bskip, Wres, bres, W1, b1, W2, b2, W3, b3,
):
    """Host-side sharding + layout packs. Returns (in_maps, flags)."""
    f32 = np.float32
    x = np.asarray(x, f32)
    condition = np.asarray(condition, f32)
    has_p = bool(
        np.any(np.asarray(Bt)) or np.any(np.asarray(Bs))
        or np.any(np.asarray(bt)) or np.any(np.asarray(bs))
    )
    has_bres = bool(np.any(np.asarray(bres)))
    has_bskip = bool(np.any(np.asarray(bskip)))

    # dtp: [core, lb=2l+br, p=16g+c, hh, t] = D_br[l, c, t, 16j+8hh+g]
    D = np.stack([np.asarray(Dt, f32), np.asarray(Ds, f32)], axis=1)
    D = D.reshape(L, 2, COND, T, 8, 2, 8)
    dtp_all = np.ascontiguousarray(
        D.transpose(4, 0, 1, 6, 2, 5, 3).reshape(NCORES, 2 * L, C, 2, T)
    ).astype(BF)
    del D

    # cstat: [16g+c, 8b+g] = condition[b, c]
    cstat = np.zeros((C, 64), f32)
    for g in range(8):
        cstat[16 * g : 16 * g + 16, g::8] = condition.T
    cstat = cstat.astype(BF)

    # wtp: [cin, (l,br,tap), cout] f32
    Wg = np.stack([np.asarray(Wt, f32), np.asarray(Ws, f32)], axis=1)
    wtp = np.ascontiguousarray(Wg.transpose(3, 0, 1, 2, 4).reshape(C, 6 * L, C))
    # wsr: [cin, (l, skip/res), cout] f32
    Ssr = np.stack([np.asarray(Wskip, f32)[:, 0], np.asarray(Wres, f32)[:, 0]], axis=1)
    wsr = np.ascontiguousarray(Ssr.transpose(2, 0, 1, 3).reshape(C, 2 * L, C))

    w1p = np.ascontiguousarray(np.asarray(W1, f32).transpose(1, 0, 2))
    b1p = np.ascontiguousarray(np.asarray(b1, f32).reshape(16, C).T)
    w2p = np.ascontiguousarray(
        np.asarray(W2, f32).reshape(3, 16, C, 2, C).transpose(2, 0, 1, 3, 4)
        .reshape(C, 96, C)
    ).astype(BF)
    b2p = np.ascontiguousarray(np.asarray(b2, f32).reshape(2, C).T)
    w3p = np.ascontiguousarray(np.asarray(W3, f32)[0, :, 0].reshape(2, C).T).astype(BF)
    b3p = np.asarray(b3, f32).reshape(1, 1)
    wcT = np.ascontiguousarray(np.asarray(Wc, f32).reshape(1, C))
    ident8 = (np.eye(C, dtype=f32) * IDENT_SCALE).astype(F8)
    bcp = np.asarray(bc, f32).reshape(C, 1)

    base = {
        "wcT": wcT, "bcp": bcp, "cstat": cstat, "wtp": wtp, "wsr": wsr,
        "w1p": w1p, "b1p": b1p, "w2p": w2p, "b2p": b2p, "w3p": w3p,
        "b3p": b3p, "ident8": ident8,
    }
    if has_p:
        P = np.stack(
            [
                np.asarray(Bt, f32) + np.asarray(bt, f32)[:, None, :],
                np.asarray(Bs, f32) + np.asarray(bs, f32)[:, None, :],
            ],
            axis=1,
        )  # [L, 2, T, C]
        P = P.reshape(L, 2, T, 8, 2, 8)
        ptp_all = np.ascontiguousarray(
            P.transpose(3, 0, 1, 4, 5, 2).reshape(NCORES, 2 * L, 16, T)
        ).astype(BF)
        del P
        pstat = np.zeros((8, 64), f32)
        for g in range(8):
            pstat[g, g::8] = 1.0
        base["pstat"] = pstat.astype(BF)
    if has_bres:
        base["bresp"] = np.ascontiguousarray(np.asarray(bres, f32).T)
    if has_bskip:
        base["bskips"] = np.asarray(bskip, f32).sum(axis=0).reshape(C, 1)

    in_maps = []
    for j in range(NCORES):
        m = dict(base)
        m["xw"] = np.ascontiguousarray(x[j, :, 0].reshape(1, T))
        m["dtp"] = dtp_all[j]
        if has_p:
            m["ptp"] = ptp_all[j]
        in_maps.append(m)
    return in_maps, (has_p, has_bres, has_bskip)


def kernel(**inputs) -> np.ndarray:
    in_maps, flags = _pack_inputs(**inputs)
    nc = _get_nc(*flags)
    res = bass_utils.run_bass_kernel_spmd(
        nc, in_maps, core_ids=list(range(NCORES))
    )
    outs = [res.results[j]["out"].reshape(T, 1) for j in range(NCORES)]
    return np.stack(outs, axis=0).astype(np.float32)
